# revision 14
# baseline (speedup 1.0000x reference)
"""Causal self-attention (B=1, T=4096, C=1024, H=8) on 8 trn2 NeuronCores.

Tensor-parallel over heads: core h owns head h (D=128 = partition width).
Everything is computed feature-major ("transposed") so the PE contraction
dim always sits on SBUF partitions.

v4: fp8 DoubleRow matmuls (2 contraction tiles per pass = 2x PE
throughput, HW-verified to stream at the same ~217ns/MM rate as bf16)
for the bulk of the work, plus a schedule that keeps the PE stream
dense:
  - Query chunk-pair 0 (tokens < 1024) stays bf16 end-to-end: max-error
    is dominated by early tokens whose softmax support is too small to
    average fp8 quantization noise. Chunk pairs 1-3 use fp8 x / fp8
    weights (x16) for QKV, and fp8 exp(att) + fp8 v (x16) for both the
    AV and the softmax-denominator matmuls.
  - Causal mask is applied on the PE: a second accumulating matmul
    (identity stationary x constant 0/-1e9 tile) onto the scores PSUM,
    keeping the scores->exp chain free of DVE round trips.
  - The Scalar engine does (almost) nothing but exp, so the
    scores->exp->AV pipeline is never blocked behind output copies.
  - c_proj of pair pc is deferred into pair pc+1's inner loop (one
    two-column-tile unit per s-tile-pair iteration) so its PSUM->SBUF
    copies and DMAs spread across the whole next pair.

Scale bookkeeping: wq/wk/wv are pre-scaled x16 (bf16 and fp8 copies), so
logits are 256x -- folded into the exp scale (free on ACT). v is stored
16x; the denominator stationary holds 16.0, so yT = (16 p@v)/(16 sum p)
comes out natural. The k-bias is dropped (softmax shift-invariance).

Per core the output partial (c_proj columns of this head only) is
written as bf16; host sums the 8 partials in f32, adds b_proj.
"""

import math
import os
import sys

for _p in ("/opt/trn_rl_repo",):
    if _p not in sys.path:
        sys.path.insert(0, _p)

import numpy as np
import ml_dtypes

import concourse.bass as bass
import concourse.mybir as mybir
import concourse.tile as tile
from concourse import bacc
from concourse import bass_utils
from concourse.masks import make_identity

B, T, C, H = 1, 4096, 1024, 8
D = C // H          # 128, head dim == partition width
N_CORES = 8
TQ = 512            # query-chunk (matmul moving free dim)
CO = C // 128      # 8 contraction tiles of 128
F32 = mybir.dt.float32
BF16 = mybir.dt.bfloat16
F8 = mybir.dt.float8e4
DR = mybir.MatmulPerfMode.DoubleRow

SW = 16.0           # weight / v scale for fp8 range
NEG = -1.0e9        # additive causal mask value

# knobs
FP8_QKV = True      # fp8 DoubleRow QKV for chunk pairs >= 1
FP8_AV = True       # fp8 DoubleRow AV + denominator for chunk pairs >= 1


def _np_dt(dt):
    return {F32: np.float32, BF16: ml_dtypes.bfloat16,
            F8: ml_dtypes.float8_e4m3}[dt]


def build(t_len=T):
    """Emit the single-core SPMD program (same code on all 8 cores)."""
    n_chunks = t_len // TQ
    n_pairs = n_chunks // 2   # query chunks processed in pairs of 2*TQ cols
    n_ttiles = t_len // 128
    T2 = 2 * TQ
    exp_scale = (1.0 / math.sqrt(D)) / (SW * SW)
    # Schraudolph exp-to-fp8 on DVE: I = max(s2*SCH_S1, 0) as uint8, where
    # the mask matmul pre-added SCH_BIAS/SCH_S1; bitcast uint8 -> e4m3.
    global SCH_S1, SCH_BIAS
    SCH_S1 = exp_scale * 8.0 / math.log(2.0)
    SCH_BIAS = 56.0 + 1.2   # +0.5 for truncating float->int conversion

    nc = bacc.Bacc(
        "TRN2", target_bir_lowering=False, debug=False, num_devices=N_CORES
    )

    # pair-0 inputs (bf16 path)
    x0T_d = nc.dram_tensor("x0T", [C, T2], BF16, kind="ExternalInput")
    wqb_d = nc.dram_tensor("wqb", [C, D], BF16, kind="ExternalInput")
    wkb_d = nc.dram_tensor("wkb", [C, D], BF16, kind="ExternalInput")
    wvb_d = nc.dram_tensor("wvb", [C, D], BF16, kind="ExternalInput")
    # pairs 1.. inputs (fp8 path); x8T holds tokens T2..t_len
    if n_pairs > 1:
        x8T_d = nc.dram_tensor("x8T", [C, t_len - T2], F8, kind="ExternalInput")
        wq8_d = nc.dram_tensor("wq8", [C, D], F8, kind="ExternalInput")
        wk8_d = nc.dram_tensor("wk8", [C, D], F8, kind="ExternalInput")
        wv8_d = nc.dram_tensor("wv8", [C, D], F8, kind="ExternalInput")
    wp_d = nc.dram_tensor("wp", [D, C], BF16, kind="ExternalInput")
    bq_d = nc.dram_tensor("bq", [D, 1], F32, kind="ExternalInput")
    bv_d = nc.dram_tensor("bv", [D, 1], F32, kind="ExternalInput")
    outP_d = nc.dram_tensor("outP", [C, t_len], BF16, kind="ExternalOutput")

    with tile.TileContext(nc) as tc:
        with (
            tc.tile_pool(name="const", bufs=1) as cpool,
            tc.tile_pool(name="persist", bufs=1) as ppool,
            tc.tile_pool(name="work", bufs=2) as wpool,
            tc.tile_pool(name="ptiles", bufs=3) as pt_pool,
            tc.tile_pool(name="psum", bufs=1, space="PSUM") as psum,
        ):
            # ---- constants / weights -------------------------------------
            # wqb first so the very first matmuls are unblocked asap
            wqb_sb = cpool.tile([128, CO, D], BF16, name="wqb_sb")
            wkb_sb = cpool.tile([128, CO, D], BF16, name="wkb_sb")
            wvb_sb = cpool.tile([128, CO, D], BF16, name="wvb_sb")
            wp_sb = cpool.tile([128, CO, D], BF16, name="wp_sb")
            nc.sync.dma_start(
                wqb_sb[:], wqb_d.ap().rearrange("(o p) m -> p o m", p=128)
            )
            bq_sb = cpool.tile([D, 1], F32, name="bq_sb")
            bv_sb = cpool.tile([D, 1], F32, name="bv_sb")
            nc.sync.dma_start(bq_sb[:], bq_d.ap())
            nc.sync.dma_start(bv_sb[:], bv_d.ap())

            # denominator stationary tiles hold 16.0 so sums = 16*sum(p);
            # memset first so the PE warmup below is unblocked immediately
            sixtb = cpool.tile([128, 128], BF16, name="sixtb")
            nc.vector.memset(sixtb[:], SW)
            # additive causal masks, applied via identity-stationary matmul:
            # maskmov[:, j, t] = NEG where t < 128*j + p. maskS8 is the
            # Schraudolph variant whose unmasked value carries the exp bias
            # (+56+c)/s1 for the DVE uint8-exp path on fp8 pairs.
            maskmov = cpool.tile([128, 4, TQ], BF16, name="maskmov")
            nc.vector.memset(maskmov[:], 0.0)
            maskS8 = cpool.tile([128, 4, TQ], BF16, name="maskS8")
            nc.vector.memset(maskS8[:], SCH_BIAS / SCH_S1)
            for j in range(4):
                for mt in (maskmov, maskS8):
                    nc.gpsimd.affine_select(
                        out=mt[:, j, :], in_=mt[:, j, :],
                        compare_op=mybir.AluOpType.is_ge, fill=NEG,
                        base=-128 * j, pattern=[[1, TQ]], channel_multiplier=-1,
                    )
            ident = cpool.tile([128, 128], BF16, name="ident")
            make_identity(nc, ident[:])
            # HAM/ifetch warmup: dummy matmuls while input DMAs land
            for wi in range(32):
                warm_ps = psum.tile([128, 128], F32, tag="s2", name="warm_ps",
                                    bufs=2)
                nc.tensor.matmul(warm_ps[:], sixtb[:], sixtb[:],
                                 start=True, stop=True)

            if n_pairs > 1:
                wq8_sb = cpool.tile([128, CO, D], F8, name="wq8_sb")
                wk8_sb = cpool.tile([128, CO, D], F8, name="wk8_sb")
                wv8_sb = cpool.tile([128, CO, D], F8, name="wv8_sb")
                sixt8 = cpool.tile([128, 2, 128], F8, name="sixt8")
                nc.vector.memset(sixt8[:], SW)

            # ---- persistent activations ----------------------------------
            kT_sb = ppool.tile([128, t_len], BF16, name="kT_sb")
            v8_sb = ppool.tile([128, n_ttiles, D], F8, name="v8_sb")
            vb_sb = ppool.tile([128, 8, D], BF16, name="vb_sb")
            yT_sb = ppool.tile([128, t_len], BF16, name="yT_sb")

            x0T_blk = x0T_d.ap().rearrange("(o p) t -> p o t", p=128)
            if n_pairs > 1:
                x8T_blk = x8T_d.ap().rearrange("(o p) t -> p o t", p=128)
            outP_blk = outP_d.ap().rearrange("(o p) t -> p o t", p=128)

            xc0 = wpool.tile([128, CO, T2], BF16, tag="xc0", name="xc0", bufs=1)
            for o in range(CO):
                nc.sync.dma_start(xc0[:, o, :], x0T_blk[:, o, :])
            for w_sb, w_d in ((wkb_sb, wkb_d), (wvb_sb, wvb_d)):
                nc.sync.dma_start(
                    w_sb[:], w_d.ap().rearrange("(o p) m -> p o m", p=128)
                )
            if n_pairs > 1:
                for w_sb, w_d in ((wq8_sb, wq8_d), (wk8_sb, wk8_d),
                                  (wv8_sb, wv8_d)):
                    nc.sync.dma_start(
                        w_sb[:], w_d.ap().rearrange("(o p) m -> p o m", p=128)
                    )
            nc.sync.dma_start(
                wp_sb[:], wp_d.ap().rearrange("d (o j) -> d o j", j=128)
            )

            def make_proj_units(pj, half):
                # c_proj of pair pj, one chunk-half, as 4 deferred units.
                # Each unit computes TWO output column tiles into one 2-bank
                # PSUM tile (single pool allocation), one PSUM->SBUF copy
                # (bf16), one DMA. Units are drained one per inner-loop
                # iteration so the copies and DMAs spread out.
                units = []
                lo = pj * T2 + half * TQ
                for j0 in range(0, CO, 2):
                    def unit(lo=lo, j0=j0, half=half):
                        oh = psum.tile([128, 2, TQ], F32, tag="s2",
                                       name="oh", bufs=2)
                        for jj in range(2):
                            nc.tensor.matmul(
                                oh[:, jj, :], wp_sb[:, j0 + jj, :],
                                yT_sb[:, lo : lo + TQ],
                                start=True, stop=True,
                            )
                        outc = wpool.tile([128, 2, TQ], BF16, tag="outc",
                                          name="outc", bufs=4)
                        if j0 in (2, 4, 6):
                            nc.scalar.copy(outc[:], oh[:])
                        else:
                            nc.vector.tensor_copy(outc[:], oh[:])
                        nc.sync.dma_start(
                            outP_d.ap()[j0 * 128 : (j0 + 2) * 128,
                                        lo : lo + TQ]
                            .rearrange("(o p) t -> p o t", p=128),
                            outc[:],
                        )
                    units.append(unit)
                return units

            pair_state = {}

            def make_qkv_units(pj, xc):
                # fp8 DoubleRow QKV for pair pj as 6 single-bank deferred
                # units (kind x half); drained inside pair pj-1's loop.
                t0p = pj * T2
                qT = wpool.tile([128, T2], BF16, tag="qT", name="qT", bufs=2)
                vT = wpool.tile([128, T2], BF16, tag="vT", name="vT", bufs=2)
                pair_state[pj] = (qT, vT)
                units = []
                for kind in range(3):   # 0=q, 1=k, 2=v
                    for half in range(2):
                        def unit(kind=kind, half=half):
                            hs = slice(half * TQ, (half + 1) * TQ)
                            dst = psum.tile([128, TQ], F32, tag="s2",
                                            name="qkvu", bufs=2)
                            w_sb = (wq8_sb, wk8_sb, wv8_sb)[kind]
                            for op in range(CO // 2):
                                o = 2 * op
                                nc.tensor.matmul(
                                    dst[:], w_sb[:, o : o + 2, :],
                                    xc[:, o : o + 2, hs],
                                    start=(op == 0), stop=(op == CO // 2 - 1),
                                    perf_mode=DR,
                                )
                            if kind == 0:
                                nc.vector.tensor_add(
                                    qT[:, hs], dst[:],
                                    bq_sb[:, 0:1].to_broadcast([D, TQ])
                                )
                            elif kind == 1:
                                nc.vector.tensor_copy(
                                    kT_sb[:, t0p + half * TQ
                                          : t0p + (half + 1) * TQ],
                                    dst[:],
                                )
                            else:
                                nc.vector.tensor_add(
                                    vT[:, hs], dst[:],
                                    bv_sb[:, 0:1].to_broadcast([D, TQ])
                                )
                        units.append(unit)
                return units

            pending = []

            def drain_one():
                if pending:
                    pending.pop(0)()

            for pc in range(n_pairs):
                t0 = pc * T2           # start of chunk A; chunk B at t0+TQ
                fp8av = pc > 0 and FP8_AV
                # next pair's x chunk DMA + its deferred QKV units
                if pc + 1 < n_pairs:
                    t0n = (pc + 1) * T2
                    xc_next = wpool.tile([128, CO, T2], F8, tag="xc",
                                         name="xc", bufs=2)
                    for o in range(CO):
                        nc.sync.dma_start(
                            xc_next[:, o, :], x8T_blk[:, o, t0n - T2 : t0n]
                        )
                    qkv_units_next = make_qkv_units(pc + 1, xc_next)
                else:
                    qkv_units_next = []

                if pc == 0:
                    # pair 0's QKV inline, bf16
                    q2 = psum.tile([128, T2], F32, tag="s2", name="q2", bufs=2)
                    k2 = psum.tile([128, T2], F32, tag="s2", name="k2", bufs=2)
                    v2 = psum.tile([128, T2], F32, tag="s2", name="v2", bufs=2)
                    for dst, w_sb in ((q2, wqb_sb), (k2, wkb_sb), (v2, wvb_sb)):
                        for o in range(CO):
                            for half in range(2):
                                hs = slice(half * TQ, (half + 1) * TQ)
                                nc.tensor.matmul(
                                    dst[:, hs], w_sb[:, o, :], xc0[:, o, hs],
                                    start=(o == 0), stop=(o == CO - 1),
                                )
                    qT_cur = wpool.tile([128, T2], BF16, tag="qT",
                                        name="qT_cur", bufs=2)
                    nc.vector.tensor_add(
                        qT_cur[:], q2[:], bq_sb[:, 0:1].to_broadcast([D, T2])
                    )
                    vT_tmp = wpool.tile([128, T2], BF16, tag="vT",
                                        name="vT_tmp", bufs=2)
                    nc.vector.tensor_add(
                        vT_tmp[:], v2[:], bv_sb[:, 0:1].to_broadcast([D, T2])
                    )
                    nc.vector.tensor_copy(kT_sb[:, t0 : t0 + T2], k2[:])
                    pair_state[0] = (qT_cur, vT_tmp)

                qT_cur, vT_tmp = pair_state[pc]

                def emit_transposes():
                    for vg in range(2):
                        vt_ps = psum.tile([128, 4, 128], BF16, tag="s2",
                                          name="vt_ps", bufs=2)
                        for tt in range(4):
                            col = (vg * 4 + tt) * 128
                            nc.tensor.transpose(
                                vt_ps[:, tt, :], vT_tmp[:, col : col + 128],
                                ident[:],
                            )
                        base = pc * 8 + vg * 4
                        nc.vector.tensor_copy(
                            v8_sb[:, base : base + 4, :], vt_ps[:]
                        )
                        if pc == 0:
                            nc.vector.tensor_copy(
                                vb_sb[:, vg * 4 : vg * 4 + 4, :], vt_ps[:]
                            )

                # ---- attention for the pair ------------------------------
                n_sA = (t0 + TQ) // 128        # s-tiles for chunk A
                n_sB = (t0 + T2) // 128        # s-tiles for chunk B
                yAB = psum.tile([128, T2], F32, tag="yAB", name="yAB", bufs=1)
                sumAB = psum.tile([128, T2], F32, tag="sumAB", name="sumAB",
                                  bufs=1)
                A, Bh = slice(0, TQ), slice(TQ, T2)
                recip = wpool.tile([128, T2], F32, tag="recip", name="recip",
                                   bufs=2)
                if pc == 0:
                    emit_transposes()   # pair 0's AV needs own v from si=0

                p_dt = F8 if fp8av else BF16
                p_tag = "p28" if fp8av else "p2b"
                n_sp = n_sB // 2

                def make_av(sp, p2):
                    # AV + denominator matmuls for step sp; emitted one
                    # iteration late (software pipelining) so the in-order PE
                    # queue never parks on an exp wait while later-emitted
                    # ready work exists.
                    si0 = 2 * sp
                    in_A = si0 < n_sA

                    def emit():
                        if fp8av:
                            for hsl, n_s, last in (
                                (A, n_sA, in_A and sp == n_sA // 2 - 1),
                                (Bh, n_sB, sp == n_sp - 1),
                            ):
                                if hsl is A and not in_A:
                                    continue
                                nc.tensor.matmul(
                                    sumAB[:, hsl], sixt8[:], p2[:, :, hsl],
                                    start=(sp == 0), stop=last, perf_mode=DR,
                                )
                                nc.tensor.matmul(
                                    yAB[:, hsl], v8_sb[:, si0 : si0 + 2, :],
                                    p2[:, :, hsl],
                                    start=(sp == 0), stop=last, perf_mode=DR,
                                )
                        else:
                            for sl_i in range(2):
                                si = si0 + sl_i
                                if in_A:
                                    nc.tensor.matmul(
                                        sumAB[:, A], sixtb[:], p2[:, sl_i, A],
                                        start=(si == 0), stop=(si == n_sA - 1),
                                    )
                                    nc.tensor.matmul(
                                        yAB[:, A], vb_sb[:, si, :],
                                        p2[:, sl_i, A],
                                        start=(si == 0), stop=(si == n_sA - 1),
                                    )
                                nc.tensor.matmul(
                                    sumAB[:, Bh], sixtb[:], p2[:, sl_i, Bh],
                                    start=(si == 0), stop=(si == n_sB - 1),
                                )
                                nc.tensor.matmul(
                                    yAB[:, Bh], vb_sb[:, si, :], p2[:, sl_i, Bh],
                                    start=(si == 0), stop=(si == n_sB - 1),
                                )
                        if in_A and si0 == n_sA - 2:
                            # A-half AV complete: normalize early, then queue
                            # the A-half c_proj to fill later iterations
                            nc.vector.reciprocal_approx_fast(recip[:, A],
                                                             sumAB[:, A])
                            nc.vector.tensor_mul(
                                yT_sb[:, t0 : t0 + TQ], yAB[:, A], recip[:, A]
                            )
                            pending.extend(make_proj_units(pc, 0))

                    return emit

                av_q = []
                for sp in range(n_sp):
                    if sp == max(0, n_sp - 7):
                        pending.extend(qkv_units_next)
                        qkv_units_next = []
                    si0 = 2 * sp
                    in_A = si0 < n_sA    # n_sA is a multiple of 4
                    p2 = pt_pool.tile([128, 2, T2], p_dt, tag=p_tag, name="p2",
                                      bufs=6 if fp8av else 4)
                    for sl_i in range(2):
                        si = si0 + sl_i
                        s0 = si * 128
                        s2 = psum.tile([128, T2], F32, tag="s2", name="s2",
                                       bufs=2)
                        diagA = in_A and si >= n_sA - 4
                        diagB = si >= n_sB - 4
                        if in_A:
                            nc.tensor.matmul(s2[:, A], kT_sb[:, s0 : s0 + 128],
                                             qT_cur[:, A], start=True,
                                             stop=not diagA)
                            if diagA:
                                nc.tensor.matmul(
                                    s2[:, A], ident[:],
                                    maskmov[:, si - (n_sA - 4), :],
                                    start=False, stop=True,
                                )
                        schrau = fp8av and not in_A
                        nc.tensor.matmul(s2[:, Bh], kT_sb[:, s0 : s0 + 128],
                                         qT_cur[:, Bh], start=True,
                                         stop=not diagB)
                        if diagB:
                            nc.tensor.matmul(
                                s2[:, Bh], ident[:],
                                (maskS8 if schrau else maskmov)
                                [:, si - (n_sB - 4), :],
                                start=False, stop=True,
                            )
                        if schrau:
                            # diagonal-region tiles: cheap exp on DVE
                            nc.vector.tensor_scalar(
                                p2[:, sl_i, Bh].bitcast(mybir.dt.uint8),
                                s2[:, Bh], SCH_S1, 0.0,
                                op0=mybir.AluOpType.mult,
                                op1=mybir.AluOpType.max,
                            )
                        else:
                            esl = slice(0, T2) if in_A else Bh
                            nc.scalar.activation(
                                p2[:, sl_i, esl], s2[:, esl],
                                mybir.ActivationFunctionType.Exp,
                                scale=exp_scale,
                            )
                    if len(av_q) >= 2:
                        av_q.pop(0)()
                    av_q.append(make_av(sp, p2))
                    drain_one()
                    if not in_A:
                        drain_one()   # diagonal region is PE-light
                    if pc > 0 and sp == 1:
                        # own-pair v only needed from si >= n_sA; transposing
                        # here hides the vT copyback latency behind scores
                        emit_transposes()

                for av in av_q:
                    av()
                av_q = []
                # normalize B first so its DVE ops aren't queued behind the
                # drained units' copies, then flush the queue (QKV of the
                # next pair, proj leftovers)
                nc.vector.reciprocal_approx_fast(recip[:, Bh], sumAB[:, Bh])
                nc.vector.tensor_mul(
                    yT_sb[:, t0 + TQ : t0 + T2], yAB[:, Bh], recip[:, Bh]
                )
                while pending:
                    drain_one()
                pending.extend(make_proj_units(pc, 1))

            # last pair's remaining c_proj
            while pending:
                drain_one()

    nc.compile()
    return nc


def make_in_maps(x, w_attn, b_attn, w_proj, b_proj, t_len=T):
    """Shard + lay out the full inputs for the 8 cores."""
    x = np.asarray(x, dtype=np.float32).reshape(t_len, C)
    w_attn = np.asarray(w_attn, dtype=np.float32)
    b_attn = np.asarray(b_attn, dtype=np.float32)
    w_proj = np.asarray(w_proj, dtype=np.float32)

    T2 = 2 * TQ
    bf = ml_dtypes.bfloat16
    f8 = ml_dtypes.float8_e4m3
    xT = np.ascontiguousarray(x.T)
    x0T = xT[:, :T2].astype(bf)
    has8 = t_len > T2
    if has8:
        x8T = np.ascontiguousarray(xT[:, T2:]).astype(f8)

    in_maps = []
    for h in range(N_CORES):
        sl = slice(h * D, (h + 1) * D)
        wq = np.ascontiguousarray((w_attn[sl, :] * SW).T)
        wk = np.ascontiguousarray((w_attn[C + h * D : C + (h + 1) * D, :] * SW).T)
        wv = np.ascontiguousarray((w_attn[2 * C + h * D : 2 * C + (h + 1) * D, :] * SW).T)
        wp = np.ascontiguousarray(w_proj[:, sl].T).astype(bf)
        m = {
            "x0T": x0T,
            "wqb": wq.astype(bf), "wkb": wk.astype(bf), "wvb": wv.astype(bf),
            "wp": wp,
            "bq": (b_attn[sl] * SW).reshape(D, 1).astype(np.float32),
            "bv": (b_attn[2 * C + h * D : 2 * C + (h + 1) * D] * SW)
                  .reshape(D, 1).astype(np.float32),
        }
        if has8:
            m["x8T"] = x8T
            m["wq8"] = wq.astype(f8)
            m["wk8"] = wk.astype(f8)
            m["wv8"] = wv.astype(f8)
        in_maps.append(m)
    return in_maps


_COMPILED = {}


def _get_compiled(t_len=T):
    if t_len not in _COMPILED:
        _COMPILED[t_len] = build(t_len)
    return _COMPILED[t_len]


def kernel(x, w_attn, b_attn, w_proj, b_proj, trace=False):
    nc = _get_compiled()
    in_maps = make_in_maps(x, w_attn, b_attn, w_proj, b_proj)
    res = bass_utils.run_bass_kernel_spmd(
        nc, in_maps, core_ids=list(range(N_CORES)), trace=trace
    )
    acc = res.results[0]["outP"].astype(np.float32)
    for h in range(1, N_CORES):
        acc += res.results[h]["outP"].astype(np.float32)
    out = acc.T + np.asarray(b_proj, dtype=np.float32)
    out = np.ascontiguousarray(out, dtype=np.float32).reshape(B, T, C)
    if trace:
        kernel.last_exec_time_ns = res.exec_time_ns
        kernel.last_results = res
    return out
